# revision 1
# baseline (speedup 1.0000x reference)
"""Trainium2 Bass kernel for nn_AutoRegerting_2954937500106.

Self-contained: builds an 8-core SPMD Bass program (tensor-parallel GRU
recurrence with per-step AllGather + V-sharded vocab head), runs it via
run_bass_kernel_spmd, and reassembles the full [B, T, V] output.
"""
import sys as _sys
for _p in ("/opt/trn_rl_repo", "/opt/trn_rl_repo/concourse"):
    if _p not in _sys.path:
        _sys.path.append(_p)

"""Bass/Tile TRN2 kernel for the 2-layer GRU + LN + vocab-head problem.

Strategy:
  - Recurrence: 8-way tensor-parallel over the gate/hidden dim. Each core
    computes a 384-row slice (128 rows of each of r/z/n) of both GRU layers'
    gate pre-activations; hidden-state chunks are exchanged every step with
    one AllGather carrying both layers' chunks; LayerNorm is computed
    redundantly on every core after the gather.
  - gi0 (input-side gates of layer 0) is precomputed batched over all T.
  - Head (Linear->LeakyReLU->LN->Linear(V)) runs batched over B*T after the
    recurrence, with the vocab projection sharded over V (4000 cols/core).
  - Layout: hidden state kept transposed ([H partitions, batch cols]) so
    gates/LN work on [128, 16] tiles and matmuls use weight-stationary form.

Host side: embedding gather, weight transposes/slicing, final concat +
transpose + b2 add.
"""
import numpy as np
import concourse.bacc as bacc
import concourse.bass as bass
import concourse.mybir as mybir
import concourse.tile as tile

F32 = mybir.dt.float32
AF = mybir.ActivationFunctionType
ALU = mybir.AluOpType

H = 1024
E = 512
B = 16
V = 32000
NCORES = 8
KH = H // 128    # 8 h-chunks
KE = E // 128    # 4 e-chunks
MSL = 3 * 128    # 384: per-core slice of the 3H gate dim
VC = V // NCORES # 4000
EPS = 1e-5
NEG_SLOPE = 0.01


def build_nc(T=256, n_cores=NCORES, rec_bf16=False):
    WDT = mybir.dt.float16 if rec_bf16 else F32
    BT = T * B
    nc = bacc.Bacc("TRN2", target_bir_lowering=False, debug=False,
                   enable_asserts=False, num_devices=n_cores)

    xT    = nc.dram_tensor("xT",    [KE, 128, BT], F32, kind="ExternalInput").ap()
    wih0  = nc.dram_tensor("wih0",  [E, MSL], F32, kind="ExternalInput").ap()
    whh0  = nc.dram_tensor("whh0",  [H, MSL], F32, kind="ExternalInput").ap()
    wih1  = nc.dram_tensor("wih1",  [H, MSL], F32, kind="ExternalInput").ap()
    whh1  = nc.dram_tensor("whh1",  [H, MSL], F32, kind="ExternalInput").ap()
    bih0c = nc.dram_tensor("bih0c", [128, 3], F32, kind="ExternalInput").ap()
    gb0   = nc.dram_tensor("gb0",   [128, 3], F32, kind="ExternalInput").ap()
    gb1   = nc.dram_tensor("gb1",   [128, 4], F32, kind="ExternalInput").ap()
    lnw   = nc.dram_tensor("lnw",   [128, 2, KH], F32, kind="ExternalInput").ap()
    lnb   = nc.dram_tensor("lnb",   [128, 2, KH], F32, kind="ExternalInput").ap()
    maskc = nc.dram_tensor("maskc", [128, KH], F32, kind="ExternalInput").ap()
    eye16 = nc.dram_tensor("eye16", [16, 16], F32, kind="ExternalInput").ap()
    ln2w  = nc.dram_tensor("ln2w",  [128, KH], F32, kind="ExternalInput").ap()
    ln2b  = nc.dram_tensor("ln2b",  [128, KH], F32, kind="ExternalInput").ap()
    b1c   = nc.dram_tensor("b1c",   [128, KH], F32, kind="ExternalInput").ap()
    w1T   = nc.dram_tensor("w1T",   [H, H], F32, kind="ExternalInput").ap()
    w2cT  = nc.dram_tensor("w2cT",  [H, VC], F32, kind="ExternalInput").ap()
    out   = nc.dram_tensor("out",   [BT, VC], F32, kind="ExternalOutput").ap()

    rg = [list(range(n_cores))]

    with tile.TileContext(nc) as tc:
        # ---- persistent DRAM scratch ----
        with tc.tile_pool(name="dramp", bufs=1, space="DRAM") as dramp:
            gi0T = dramp.tile([3, 128, BT], F32)
            h1T  = dramp.tile([KH, 128, BT], F32)
            aTn  = dramp.tile([KH, 128, BT], F32)

            # ================= Phase 0: batched gi0 =================
            with tc.tile_pool(name="p0", bufs=1) as p0, \
                 tc.tile_pool(name="p0w", bufs=2) as p0w, \
                 tc.tile_pool(name="ps0", bufs=2, space="PSUM") as ps0:
                xs = p0.tile([128, KE, BT], F32)
                nc.sync.dma_start(xs[:], xT.rearrange("k p n -> p k n"))
                wih0s = p0.tile([128, KE, MSL], F32)
                nc.sync.dma_start(wih0s[:], wih0.rearrange("(k p) m -> p k m", p=128))
                bih0s = p0.tile([128, 3], F32)
                nc.sync.dma_start(bih0s[:], bih0c[:])
                nbt = (BT + 511) // 512
                assert BT % nbt == 0
                btc = BT // nbt
                for m in range(3):
                    for j in range(nbt):
                        ps = ps0.tile([128, btc], F32, tag="ps")
                        for k in range(KE):
                            nc.tensor.matmul(
                                ps[:],
                                wih0s[:, k, m * 128:(m + 1) * 128],
                                xs[:, k, j * btc:(j + 1) * btc],
                                start=(k == 0), stop=(k == KE - 1))
                        gtile = p0w.tile([128, btc], F32, tag="gt")
                        nc.scalar.activation(gtile[:], ps[:], AF.Identity,
                                             bias=bih0s[:, m:m + 1])
                        nc.sync.dma_start(gi0T[m, :, j * btc:(j + 1) * btc], gtile[:])

            # ================= Phase 1: recurrence =================
            with tc.tile_pool(name="rw", bufs=1) as rw, \
                 tc.tile_pool(name="rs", bufs=2) as rs, \
                 tc.tile_pool(name="rtmp", bufs=3) as rtmp, \
                 tc.tile_pool(name="rps", bufs=1, space="PSUM") as rps, \
                 tc.tile_pool(name="rpst", bufs=1, space="PSUM") as rpst, \
                 tc.tile_pool(name="rdram", bufs=3, space="DRAM") as rdram:

                whh0s = rw.tile([128, KH, MSL], WDT)
                wih1s = rw.tile([128, KH, MSL], WDT)
                whh1s = rw.tile([128, KH, MSL], WDT)
                with tc.tile_pool(name="wstg", bufs=2) as wstg:
                    for wsrc, wdst in ((whh0, whh0s), (wih1, wih1s),
                                       (whh1, whh1s)):
                        wtmp = wstg.tile([128, KH, MSL], F32, tag="wtmp",
                                         name="wtmp")
                        nc.sync.dma_start(
                            wtmp[:], wsrc.rearrange("(k p) m -> p k m", p=128))
                        nc.vector.tensor_copy(wdst[:], wtmp[:])
                gb0s = rw.tile([128, 3], F32)
                nc.sync.dma_start(gb0s[:], gb0[:])
                gb1s = rw.tile([128, 4], F32)
                nc.sync.dma_start(gb1s[:], gb1[:])
                lnws = rw.tile([128, 2, KH], F32)
                nc.sync.dma_start(lnws[:], lnw[:])
                lnbs = rw.tile([128, 2, KH], F32)
                nc.sync.dma_start(lnbs[:], lnb[:])
                masks = rw.tile([128, KH], F32)
                nc.sync.dma_start(masks[:], maskc[:])
                eyes = rw.tile([16, 16], F32)
                nc.sync.dma_start(eyes[:], eye16[:])
                ones_col = rw.tile([128, 1], F32)
                nc.vector.memset(ones_col[:], 1.0)
                ones_row = rw.tile([1, 128], F32)
                nc.vector.memset(ones_row[:], 1.0)

                for k in range(T + 2):
                    last = (k == T + 1)
                    # ---- LN of AG#(k-1) -> h0[k-1], h1[k-2] ----
                    hn = rs.tile([128, 2, KH, B], F32, tag="hn")
                    if k == 0:
                        nc.vector.memset(hn[:], 0.0)
                    else:
                        hp = rtmp.tile([128, 2, KH, B], F32, tag="hp")
                        ag4 = agout[:].rearrange("(c l p) b -> p l c b", p=128, l=2)
                        nc.sync.dma_start(hp[:, 0], ag4[:, 0])
                        nc.sync.dma_start(hp[:, 1], ag4[:, 1])
                        sq = rtmp.tile([128, 2, KH, B], F32, tag="sq")
                        nc.scalar.activation(sq[:], hp[:], AF.Square)
                        ps_s = rpst.tile([1, 2, B, KH], F32, tag="pstat")
                        nc.tensor.matmul(ps_s[:], ones_col[:],
                                         hp.rearrange("p l c b -> p l b c"),
                                         start=True, stop=True)
                        ps_q = rpst.tile([1, 2, B, KH], F32, tag="pstat")
                        nc.tensor.matmul(ps_q[:], ones_col[:],
                                         sq.rearrange("p l c b -> p l b c"),
                                         start=True, stop=True)
                        mean = rtmp.tile([1, 2, B], F32, tag="mean")
                        nc.vector.tensor_reduce(mean[:], ps_s[:],
                                                mybir.AxisListType.X, ALU.add)
                        nc.vector.tensor_scalar_mul(mean[:], mean[:], 1.0 / H)
                        msq = rtmp.tile([1, 2, B], F32, tag="msq")
                        nc.vector.tensor_reduce(msq[:], ps_q[:],
                                                mybir.AxisListType.X, ALU.add)
                        nc.vector.tensor_scalar_mul(msq[:], msq[:], 1.0 / H)
                        var = rtmp.tile([1, 2, B], F32, tag="var")
                        rstd = rtmp.tile([1, 2, B], F32, tag="rstd")
                        nc.vector.tensor_mul(var[:], mean[:], mean[:])
                        nc.vector.tensor_sub(var[:], msq[:], var[:])
                        nc.vector.tensor_scalar_add(var[:], var[:], EPS)
                        nc.vector.reciprocal(var[:], var[:])
                        nc.scalar.activation(rstd[:], var[:], AF.Sqrt)
                        ps_mb = rpst.tile([128, 2, KH, B], F32, tag="pstat")
                        nc.tensor.matmul(
                            ps_mb[:], ones_row[:],
                            mean.unsqueeze(2).broadcast_to([1, 2, KH, B]),
                            start=True, stop=True)
                        ps_rb = rpst.tile([128, 2, KH, B], F32, tag="pstat")
                        nc.tensor.matmul(
                            ps_rb[:], ones_row[:],
                            rstd.unsqueeze(2).broadcast_to([1, 2, KH, B]),
                            start=True, stop=True)
                        nc.vector.tensor_sub(hn[:], hp[:], ps_mb[:])
                        nc.vector.tensor_mul(hn[:], hn[:], ps_rb[:])
                        nc.vector.tensor_mul(
                            hn[:], hn[:],
                            lnws.unsqueeze(3).broadcast_to([128, 2, KH, B]))
                        nc.vector.tensor_add(
                            hn[:], hn[:],
                            lnbs.unsqueeze(3).broadcast_to([128, 2, KH, B]))
                        if k == 1:
                            nc.vector.memset(hn[:, 1], 0.0)
                    hnb = rs.tile([128, 2, KH, B], WDT, tag="hnb")
                    nc.vector.tensor_copy(hnb[:], hn[:])
                    # store h1[k-2]
                    if 2 <= k <= T + 1:
                        nc.sync.dma_start(
                            h1T[:, :, (k - 2) * B:(k - 1) * B].rearrange(
                                "c p b -> p c b"),
                            hn[:, 1])
                    if last:
                        break

                    # ---- gate matmuls (h-stationary, stream weights) ----
                    pA0 = rps.tile([16, MSL], F32, tag="pA0")
                    pA1i = rps.tile([16, MSL], F32, tag="pA1i")
                    pA1h = rps.tile([16, MSL], F32, tag="pA1h")
                    h0a = hn[:, 0]
                    h1a = hn[:, 1]
                    for kk in range(KH):
                        nc.tensor.matmul(pA0[:], h0a[:, kk, :], whh0s[:, kk, :],
                                         start=(kk == 0), stop=(kk == KH - 1))
                    for kk in range(KH):
                        nc.tensor.matmul(pA1i[:], h0a[:, kk, :], wih1s[:, kk, :],
                                         start=(kk == 0), stop=(kk == KH - 1))
                    for kk in range(KH):
                        nc.tensor.matmul(pA1h[:], h1a[:, kk, :], whh1s[:, kk, :],
                                         start=(kk == 0), stop=(kk == KH - 1))
                    sA0 = rtmp.tile([16, MSL], F32, tag="sA0")
                    nc.vector.tensor_copy(sA0[:], pA0[:])
                    sA1i = rtmp.tile([16, MSL], F32, tag="sA1i")
                    nc.vector.tensor_copy(sA1i[:], pA1i[:])
                    sA1h = rtmp.tile([16, MSL], F32, tag="sA1h")
                    nc.vector.tensor_copy(sA1h[:], pA1h[:])
                    sA1rz = rtmp.tile([16, 256], F32, tag="sA1rz")
                    nc.vector.tensor_add(sA1rz[:], sA1i[:, 0:256], sA1h[:, 0:256])
                    # transpose to [128,16] gate tiles
                    pTa = rps.tile([128, 2, B], F32, tag="pTa")
                    pTb = rps.tile([128, 2, B], F32, tag="pTb")
                    pTc = rps.tile([128, 2, B], F32, tag="pTc")
                    pTd = rps.tile([128, B], F32, tag="pTd")
                    nc.tensor.transpose(pTa[:, 0], sA0[:, 0:128], eyes[:])
                    nc.tensor.transpose(pTa[:, 1], sA0[:, 128:256], eyes[:])
                    nc.tensor.transpose(pTb[:, 0], sA0[:, 256:384], eyes[:])
                    nc.tensor.transpose(pTb[:, 1], sA1i[:, 256:384], eyes[:])
                    nc.tensor.transpose(pTc[:, 0], sA1rz[:, 0:128], eyes[:])
                    nc.tensor.transpose(pTc[:, 1], sA1rz[:, 128:256], eyes[:])
                    nc.tensor.transpose(pTd[:], sA1h[:, 256:384], eyes[:])
                    bank_a = pTa
                    bank_c = pTc
                    bank_d = pTd

                    # ---- gi0 slice for this tick ----
                    tgi = min(k, T - 1)
                    gi0t = rtmp.tile([128, 3, B], F32, tag="gi0t")
                    nc.sync.dma_start(
                        gi0t[:], gi0T[:, :, tgi * B:(tgi + 1) * B].rearrange(
                            "m p b -> p m b"))

                    # ---- layer0 gates ----
                    t0r = rtmp.tile([128, B], F32, tag="t0r")
                    nc.vector.scalar_tensor_tensor(t0r[:], bank_a[:, 0],
                                                   gb0s[:, 0:1], gi0t[:, 0],
                                                   ALU.add, ALU.add)
                    r0 = rtmp.tile([128, B], F32, tag="r0")
                    nc.scalar.activation(r0[:], t0r[:], AF.Sigmoid)
                    t0z = rtmp.tile([128, B], F32, tag="t0z")
                    nc.vector.scalar_tensor_tensor(t0z[:], bank_a[:, 1],
                                                   gb0s[:, 1:2], gi0t[:, 1],
                                                   ALU.add, ALU.add)
                    z0 = rtmp.tile([128, B], F32, tag="z0")
                    nc.scalar.activation(z0[:], t0z[:], AF.Sigmoid)
                    hn0m = rtmp.tile([128, B], F32, tag="hn0m")
                    nc.vector.tensor_scalar_add(hn0m[:], pTb[:, 0], gb0s[:, 2:3])
                    nc.vector.tensor_mul(hn0m[:], hn0m[:], r0[:])
                    nc.vector.tensor_add(hn0m[:], hn0m[:], gi0t[:, 2])
                    n0 = rtmp.tile([128, B], F32, tag="n0")
                    nc.scalar.activation(n0[:], hn0m[:], AF.Tanh)
                    # h chunk select (this core's rows of h0[k-1])
                    hsel0 = rtmp.tile([128, KH, B], F32, tag="hsel0")
                    nc.vector.tensor_mul(
                        hsel0[:], hn[:, 0],
                        masks.unsqueeze(2).broadcast_to([128, KH, B]))
                    hc0 = rtmp.tile([128, B], F32, tag="hc0")
                    nc.vector.tensor_reduce(
                        hc0[:], hsel0.rearrange("p c b -> p b c"),
                        mybir.AxisListType.X, ALU.add)
                    h0p = rtmp.tile([128, B], F32, tag="h0p")
                    nc.vector.tensor_sub(h0p[:], hc0[:], n0[:])
                    nc.vector.tensor_mul(h0p[:], h0p[:], z0[:])
                    nc.vector.tensor_add(h0p[:], h0p[:], n0[:])

                    # ---- layer1 gates ----
                    r1 = rtmp.tile([128, B], F32, tag="r1")
                    nc.scalar.activation(r1[:], bank_c[:, 0], AF.Sigmoid,
                                         bias=gb1s[:, 0:1])
                    z1 = rtmp.tile([128, B], F32, tag="z1")
                    nc.scalar.activation(z1[:], bank_c[:, 1], AF.Sigmoid,
                                         bias=gb1s[:, 1:2])
                    hn1m = rtmp.tile([128, B], F32, tag="hn1m")
                    nc.vector.tensor_scalar_add(hn1m[:], bank_d[:], gb1s[:, 3:4])
                    nc.vector.tensor_mul(hn1m[:], hn1m[:], r1[:])
                    nc.vector.scalar_tensor_tensor(hn1m[:], pTb[:, 1],
                                                   gb1s[:, 2:3], hn1m[:],
                                                   ALU.add, ALU.add)
                    n1 = rtmp.tile([128, B], F32, tag="n1")
                    nc.scalar.activation(n1[:], hn1m[:], AF.Tanh)
                    hsel1 = rtmp.tile([128, KH, B], F32, tag="hsel1")
                    nc.vector.tensor_mul(
                        hsel1[:], hn[:, 1],
                        masks.unsqueeze(2).broadcast_to([128, KH, B]))
                    hc1 = rtmp.tile([128, B], F32, tag="hc1")
                    nc.vector.tensor_reduce(
                        hc1[:], hsel1.rearrange("p c b -> p b c"),
                        mybir.AxisListType.X, ALU.add)
                    h1p = rtmp.tile([128, B], F32, tag="h1p")
                    nc.vector.tensor_sub(h1p[:], hc1[:], n1[:])
                    nc.vector.tensor_mul(h1p[:], h1p[:], z1[:])
                    nc.vector.tensor_add(h1p[:], h1p[:], n1[:])

                    # ---- AllGather ----
                    agin = rdram.tile([2, 128, B], F32, tag="agin")
                    nc.sync.dma_start(agin[0], h0p[:])
                    nc.sync.dma_start(agin[1], h1p[:])
                    agout = rdram.tile([n_cores * 2 * 128, B], F32, tag="agout",
                                       addr_space="Shared")
                    nc.gpsimd.collective_compute(
                        "AllGather", ALU.bypass, replica_groups=rg,
                        ins=[agin.opt()], outs=[agout.opt()])

            # ================= Phase 2a: a = LN(LeakyReLU(h1 @ W1.T + b1)) ====
            with tc.tile_pool(name="aw", bufs=1) as aw, \
                 tc.tile_pool(name="ah", bufs=2) as ah, \
                 tc.tile_pool(name="atmp", bufs=2) as atmp, \
                 tc.tile_pool(name="aout", bufs=3) as aout, \
                 tc.tile_pool(name="aps", bufs=2, space="PSUM") as aps, \
                 tc.tile_pool(name="apst", bufs=2, space="PSUM") as apst:
                w1s = aw.tile([128, KH, H], F32)
                nc.sync.dma_start(w1s[:], w1T.rearrange("(k p) m -> p k m", p=128))
                b1s = aw.tile([128, KH], F32)
                nc.sync.dma_start(b1s[:], b1c[:])
                ln2ws = aw.tile([128, KH], F32)
                nc.sync.dma_start(ln2ws[:], ln2w[:])
                ln2bs = aw.tile([128, KH], F32)
                nc.sync.dma_start(ln2bs[:], ln2b[:])
                ones_col2 = aw.tile([128, 1], F32)
                nc.vector.memset(ones_col2[:], 1.0)
                ones_row2 = aw.tile([1, 128], F32)
                nc.vector.memset(ones_row2[:], 1.0)

                nbt2 = (BT + 511) // 512
                assert BT % nbt2 == 0
                btc2 = BT // nbt2
                for j in range(nbt2):
                    hk = ah.tile([128, KH, btc2], F32, tag="hk")
                    nc.sync.dma_start(
                        hk[:], h1T[:, :, j * btc2:(j + 1) * btc2].rearrange(
                            "k p n -> p k n"))
                    atiles = []
                    ps_s2 = apst.tile([1, btc2], F32, tag="pstat2")
                    ps_q2 = apst.tile([1, btc2], F32, tag="pstat2")
                    for m in range(KH):
                        ps_a = aps.tile([128, btc2], F32, tag="ps_a")
                        for kk in range(KH):
                            nc.tensor.matmul(ps_a[:],
                                             w1s[:, kk, m * 128:(m + 1) * 128],
                                             hk[:, kk, :],
                                             start=(kk == 0), stop=(kk == KH - 1))
                        rl = aout.tile([128, btc2], F32, tag="rl")
                        nc.scalar.activation(rl[:], ps_a[:], AF.Relu,
                                             bias=b1s[:, m:m + 1])
                        at = atmp.tile([128, btc2], F32, tag=f"at{m}")
                        # leaky_relu(y) = alpha*(y - relu(y)) + relu(y)
                        nc.vector.scalar_tensor_tensor(
                            at[:], ps_a[:], b1s[:, m:m + 1], rl[:],
                            ALU.add, ALU.subtract)
                        nc.vector.scalar_tensor_tensor(
                            at[:], at[:], NEG_SLOPE, rl[:], ALU.mult, ALU.add)
                        atiles.append(at)
                        nc.tensor.matmul(ps_s2[:], ones_col2[:], at[:],
                                         start=(m == 0), stop=(m == KH - 1))
                        sq2 = aout.tile([128, btc2], F32, tag="sq2")
                        nc.scalar.activation(sq2[:], at[:], AF.Square)
                        nc.tensor.matmul(ps_q2[:], ones_col2[:], sq2[:],
                                         start=(m == 0), stop=(m == KH - 1))
                    mean2 = atmp.tile([1, btc2], F32, tag="mean2")
                    nc.vector.tensor_scalar_mul(mean2[:], ps_s2[:], 1.0 / H)
                    var2 = atmp.tile([1, btc2], F32, tag="var2")
                    nc.vector.tensor_mul(var2[:], mean2[:], mean2[:])
                    nc.vector.scalar_tensor_tensor(var2[:], ps_q2[:], 1.0 / H,
                                                   var2[:], ALU.mult, ALU.subtract)
                    nc.vector.tensor_scalar_add(var2[:], var2[:], EPS)
                    nc.vector.reciprocal(var2[:], var2[:])
                    rstd2 = atmp.tile([1, btc2], F32, tag="rstd2")
                    nc.scalar.activation(rstd2[:], var2[:], AF.Sqrt)
                    ps_mb2 = apst.tile([128, btc2], F32, tag="pstat2")
                    nc.tensor.matmul(ps_mb2[:], ones_row2[:],
                                     mean2[:], start=True, stop=True)
                    ps_rb2 = apst.tile([128, btc2], F32, tag="pstat2")
                    nc.tensor.matmul(ps_rb2[:], ones_row2[:],
                                     rstd2[:], start=True, stop=True)
                    for m in range(KH):
                        at = atiles[m]
                        an = aout.tile([128, btc2], F32, tag="an")
                        nc.vector.tensor_sub(an[:], at[:], ps_mb2[:])
                        nc.vector.tensor_mul(an[:], an[:], ps_rb2[:])
                        nc.vector.tensor_scalar(an[:], an[:], ln2ws[:, m:m + 1],
                                                ln2bs[:, m:m + 1],
                                                ALU.mult, ALU.add)
                        nc.sync.dma_start(aTn[m, :, j * btc2:(j + 1) * btc2], an[:])

            # ================= Phase 2b: logits = aTn.T @ W2cT =================
            with tc.tile_pool(name="lw", bufs=1) as lw, \
                 tc.tile_pool(name="la", bufs=2) as la, \
                 tc.tile_pool(name="lo", bufs=3) as lo, \
                 tc.tile_pool(name="lps", bufs=1, space="PSUM") as lps:
                w2s = lw.tile([128, KH, VC], F32)
                nc.sync.dma_start(w2s[:], w2cT.rearrange("(k p) v -> p k v", p=128))
                nvc = 8
                vcs = VC // nvc  # 500
                mtw = min(128, BT)
                for mt in range(BT // mtw):
                    ast = la.tile([128, KH, mtw], F32, tag="ast")
                    nc.sync.dma_start(
                        ast[:], aTn[:, :, mt * mtw:(mt + 1) * mtw].rearrange(
                            "k p n -> p k n"))
                    pvs = [lps.tile([mtw, vcs], F32, tag=f"pv{v}", name=f"pv{v}")
                           for v in range(nvc)]
                    for kk in range(KH):
                        for v in range(nvc):
                            nc.tensor.matmul(pvs[v][:], ast[:, kk, :mtw],
                                             w2s[:, kk, v * vcs:(v + 1) * vcs],
                                             start=(kk == 0), stop=(kk == KH - 1))
                    for v in range(nvc):
                        ot = lo.tile([mtw, vcs], F32, tag="ot")
                        if v % 2 == 0:
                            nc.vector.tensor_copy(ot[:], pvs[v][:])
                        else:
                            nc.scalar.copy(ot[:], pvs[v][:])
                        nc.sync.dma_start(
                            out[mt * mtw:(mt + 1) * mtw, v * vcs:(v + 1) * vcs],
                            ot[:])
    return nc


# ===================== host-side prep / post =====================

def _np(x):
    return np.asarray(x)


def prep_in_maps(inputs, T=256, n_cores=NCORES):
    """inputs: dict from setup_inputs() (numpy). Returns list of in_maps."""
    ids = _np(inputs['input']).astype(np.int64)[:, :T]          # [B, T]
    embd = _np(inputs['embd']).astype(np.float32)               # [V, E]
    BT = T * B
    # xT: [KE, 128, BT] with column index t*16+b
    x = embd[ids]                                               # [B, T, E]
    xT = np.ascontiguousarray(x.transpose(2, 1, 0).reshape(E, T * B))  # [E, (t b)]
    xT = xT.reshape(KE, 128, BT)

    def gate_slice(W, c):
        # W: [3H, D] -> per-core [D, 384] with cols (r,z,n) x 128
        cols = []
        for g in range(3):
            cols.append(W[g * H + c * 128:(g * H + (c + 1) * 128), :])  # [128, D]
        Wc = np.concatenate(cols, axis=0)                        # [384, D]
        return np.ascontiguousarray(Wc.T)                        # [D, 384]

    def bias_slice(b, c, g):
        return b[g * H + c * 128:g * H + (c + 1) * 128]

    lnw = np.stack([_np(inputs['ln0_w']), _np(inputs['ln1_w'])], 0)  # [2, H]
    lnb = np.stack([_np(inputs['ln0_b']), _np(inputs['ln1_b'])], 0)
    lnw_t = np.ascontiguousarray(
        lnw.reshape(2, KH, 128).transpose(2, 0, 1)).astype(np.float32)
    lnb_t = np.ascontiguousarray(
        lnb.reshape(2, KH, 128).transpose(2, 0, 1)).astype(np.float32)
    ln2w_t = np.ascontiguousarray(
        _np(inputs['ln2_w']).reshape(KH, 128).T).astype(np.float32)
    ln2b_t = np.ascontiguousarray(
        _np(inputs['ln2_b']).reshape(KH, 128).T).astype(np.float32)
    b1_t = np.ascontiguousarray(
        _np(inputs['b1']).reshape(KH, 128).T).astype(np.float32)
    w1T = np.ascontiguousarray(_np(inputs['W1']).astype(np.float32).T)  # [H, H]
    W2 = _np(inputs['W2']).astype(np.float32)

    Wih0 = _np(inputs['Wih0']).astype(np.float32)
    Whh0 = _np(inputs['Whh0']).astype(np.float32)
    Wih1 = _np(inputs['Wih1']).astype(np.float32)
    Whh1 = _np(inputs['Whh1']).astype(np.float32)
    bih0 = _np(inputs['bih0']).astype(np.float32)
    bhh0 = _np(inputs['bhh0']).astype(np.float32)
    bih1 = _np(inputs['bih1']).astype(np.float32)
    bhh1 = _np(inputs['bhh1']).astype(np.float32)

    in_maps = []
    for c in range(n_cores):
        bih0c = np.stack([bias_slice(bih0, c, g) for g in range(3)], 1)  # [128,3]
        gb0 = np.stack([bias_slice(bhh0, c, g) for g in range(3)], 1)
        gb1 = np.stack([
            bias_slice(bih1, c, 0) + bias_slice(bhh1, c, 0),
            bias_slice(bih1, c, 1) + bias_slice(bhh1, c, 1),
            bias_slice(bih1, c, 2),
            bias_slice(bhh1, c, 2)], 1)                                  # [128,4]
        maskc = np.zeros((128, KH), np.float32)
        maskc[:, c] = 1.0
        eye16 = np.eye(16, dtype=np.float32)
        w2cT = np.ascontiguousarray(W2[c * VC:(c + 1) * VC, :].T)        # [H, VC]
        in_maps.append({
            'xT': xT, 'wih0': gate_slice(Wih0, c), 'whh0': gate_slice(Whh0, c),
            'wih1': gate_slice(Wih1, c), 'whh1': gate_slice(Whh1, c),
            'bih0c': np.ascontiguousarray(bih0c),
            'gb0': np.ascontiguousarray(gb0), 'gb1': np.ascontiguousarray(gb1),
            'lnw': lnw_t, 'lnb': lnb_t, 'maskc': maskc, 'eye16': eye16,
            'ln2w': ln2w_t, 'ln2b': ln2b_t, 'b1c': b1_t,
            'w1T': w1T, 'w2cT': w2cT,
        })
    return in_maps


def postprocess(results, inputs, T=256):
    """results: list of per-core {'out': [BT, VC]}. Returns [B, T, V]."""
    b2 = _np(inputs['b2']).astype(np.float32)
    full = np.concatenate([r['out'] for r in results], axis=1)  # [BT, V]
    full = full.reshape(T, B, V).transpose(1, 0, 2)             # [B, T, V]
    return full + b2


# ===================== numpy mirror (for sim testing) =====================

def numpy_reference(inputs, T=256):
    ids = _np(inputs['input']).astype(np.int64)[:, :T]
    embd = _np(inputs['embd'])
    x = embd[ids].astype(np.float32)        # [B, T, E]
    h0 = np.zeros((B, H), np.float32)
    h1 = np.zeros((B, H), np.float32)

    def ln(v, w, bb):
        m = v.mean(-1, keepdims=True)
        var = v.var(-1, keepdims=True)
        return (v - m) / np.sqrt(var + EPS) * w + bb

    def gru(xx, hh, Wih, Whh, bih, bhh):
        gi = xx @ _np(Wih).T + _np(bih)
        gh = hh @ _np(Whh).T + _np(bhh)
        ir, iz, inn = np.split(gi, 3, -1)
        hr, hz, hn_ = np.split(gh, 3, -1)
        r = 1 / (1 + np.exp(-(ir + hr)))
        z = 1 / (1 + np.exp(-(iz + hz)))
        n = np.tanh(inn + r * hn_)
        return (1 - z) * n + z * hh

    outs = []
    for t in range(T):
        h0 = ln(gru(x[:, t], h0, inputs['Wih0'], inputs['Whh0'],
                    inputs['bih0'], inputs['bhh0']),
                _np(inputs['ln0_w']), _np(inputs['ln0_b']))
        h1 = ln(gru(h0, h1, inputs['Wih1'], inputs['Whh1'],
                    inputs['bih1'], inputs['bhh1']),
                _np(inputs['ln1_w']), _np(inputs['ln1_b']))
        a = h1 @ _np(inputs['W1']).T + _np(inputs['b1'])
        a = np.where(a > 0, a, NEG_SLOPE * a)
        a = ln(a, _np(inputs['ln2_w']), _np(inputs['ln2_b']))
        outs.append(a @ _np(inputs['W2']).T + _np(inputs['b2']))
    return np.stack(outs, 1)  # [B, T, V]


# ===================== NEFF disk cache =====================

def _install_neff_cache():
    import hashlib, os, shutil
    import concourse.bass2jax as b2j
    from concourse.bass_utils import compile_bir_kernel as _real
    if getattr(b2j, "_ant_neff_cache_installed", False):
        return
    cache_dir = os.path.expanduser("~/.cache/bass_neff_cache")
    os.makedirs(cache_dir, exist_ok=True)

    def cached(bir_json, tmpdir, neff_name="file.neff"):
        key = hashlib.sha256(bir_json).hexdigest()
        p = os.path.join(cache_dir, key + ".neff")
        out = os.path.join(tmpdir, neff_name)
        if os.path.exists(p):
            shutil.copyfile(p, out)
            return out
        r = _real(bir_json, tmpdir, neff_name)
        try:
            shutil.copyfile(r, p)
        except OSError:
            pass
        return r

    b2j.compile_bir_kernel = cached
    b2j._ant_neff_cache_installed = True


# ===================== NTFF profile shim (for traced runs) ==================

def _install_axon_prof():
    import types, ctypes, contextlib
    try:
        from antenv import axon_hooks  # noqa: F401
        return
    except ImportError:
        pass
    so_path = "/opt/axon/libaxon_pjrt.so"
    try:
        lib = ctypes.CDLL(so_path)
    except OSError:
        return
    hook = None
    if hasattr(lib, "axon_start_nrt_profile"):
        lib.axon_start_nrt_profile.argtypes = [
            ctypes.POINTER(ctypes.c_int64), ctypes.c_size_t]
        lib.axon_start_nrt_profile.restype = ctypes.c_int64
        lib.axon_stop_nrt_profile.argtypes = [ctypes.c_char_p]
        lib.axon_stop_nrt_profile.restype = ctypes.c_int64

        @contextlib.contextmanager
        def hook(output_dir, device_ids):
            import jax
            jax.devices()
            if device_ids:
                ids = (ctypes.c_int64 * len(device_ids))(*device_ids)
                rc = lib.axon_start_nrt_profile(ids, len(device_ids))
            else:
                rc = lib.axon_start_nrt_profile(None, 0)
            if rc != 0:
                raise RuntimeError(f"axon_start_nrt_profile rc={rc}")
            try:
                yield
            finally:
                lib.axon_stop_nrt_profile(str(output_dir).encode())

    mod = types.ModuleType("antenv.axon_hooks")
    _h = [hook]
    mod.set_axon_ntff_profile_hook = lambda h: _h.__setitem__(0, h)
    mod.get_axon_ntff_profile_hook = lambda: _h[0]
    _sys.modules["antenv.axon_hooks"] = mod
    import antenv
    antenv.axon_hooks = mod


# ===================== entry point =====================

_NC = None


def _get_nc():
    global _NC
    if _NC is None:
        _install_neff_cache()
        nc = build_nc(T=256)
        nc.compile()
        _NC = nc
    return _NC


def kernel(**inputs):
    import numpy as np
    from concourse import bass_utils
    nc = _get_nc()
    in_maps = prep_in_maps(inputs, T=256)
    res = bass_utils.run_bass_kernel_spmd(
        nc, in_maps, core_ids=list(range(NCORES)))
    return postprocess(res.results, inputs, T=256)


def kernel_traced(**inputs):
    """Like kernel() but also returns neuron-profile exec_time_ns."""
    from concourse import bass_utils
    _install_axon_prof()
    nc = _get_nc()
    in_maps = prep_in_maps(inputs, T=256)
    res = bass_utils.run_bass_kernel_spmd(
        nc, in_maps, core_ids=list(range(NCORES)), trace=True)
    return postprocess(res.results, inputs, T=256), res.exec_time_ns



# revision 11
# speedup vs baseline: 1.5816x; 1.5816x over previous
"""Trainium2 Bass kernel for nn_AutoRegerting_2954937500106.

Self-contained: builds an 8-core SPMD Bass program and reassembles the
full [B, T, V] output.

Strategy (v2):
  - Recurrence: 8-way tensor-parallel over the gate dim, with LayerNorm
    FOLDED into the gate matmuls: each core ships its raw-h slice
    (u = lnw*h_raw) plus bn_stats partial stats in the AllGather; the
    consumer corrects the matmul output with rstd/mean via rank-1 terms
    (vw = W @ lnw row-sums precomputed on host).  Gates are computed in
    batch-major [16, 384] layout (no transposes, no stat matmuls, no
    partition broadcasts, no sqrt-activation table churn - rstd comes
    from a Newton rsqrt on gpsimd).
  - Two AllGathers per step (one per GRU layer) so the two layers'
    dependency loops pipeline.
  - Head (Linear->LeakyReLU->LN->Linear(V)) in bf16, V-sharded
    (4000 cols/core), reading u1/stats straight out of the persistent
    AllGather output buffers; LN1 is folded into the W1 matmul via two
    extra contraction rows (-mean, std) and LeakyReLU positive
    homogeneity + LN2 scale invariance.
  - gi0 (input-side gates of layer 0) precomputed batched over T in
    fp32 (feeds the recurrence; chaotic sensitivity forbids bf16 there).
"""
import sys as _sys
for _p in ("/opt/trn_rl_repo", "/opt/trn_rl_repo/concourse"):
    if _p not in _sys.path:
        _sys.path.append(_p)

import numpy as np
import concourse.bacc as bacc
import concourse.bass as bass
import concourse.mybir as mybir
import concourse.tile as tile

F32 = mybir.dt.float32
BF16 = mybir.dt.bfloat16
I32 = mybir.dt.int32
AF = mybir.ActivationFunctionType
ALU = mybir.AluOpType

H = 1024
E = 512
B = 16
V = 32000
T = 256
BT = T * B
NCORES = 8
KH = H // 128     # 8 h-chunks
KE = E // 128     # 4 e-chunks
MSL = 3 * 128     # 384: per-core slice of the 3H gate dim
VC = V // NCORES  # 4000
EPS = 1e-5
NEG_SLOPE = 0.01
QK = 0x5F3759DF   # quake rsqrt seed constant
PAY = 128 * 16 + 16 * 6   # AG payload floats: u [128,16] + stats [16,6] b-major


def _rsqrt(nc, pool, x, ktile, one_i, cts, tag):
    """rstd = 1/sqrt(x): quake seed + 3 Newton iterations, gpsimd
    tensor_tensor only (Pool rejects TensorScalarPtr)."""
    g = nc.gpsimd
    c15, neghalf = cts
    j = pool.tile([16, 1], I32, tag=f"{tag}j")
    nc.vector.tensor_tensor(j[:], x[:].bitcast(I32), one_i[:],
                            ALU.logical_shift_right)
    y = pool.tile([16, 1], F32, tag=f"{tag}y")
    g.tensor_tensor(y[:].bitcast(I32), ktile[:], j[:], ALU.subtract)
    t1 = pool.tile([16, 1], F32, tag=f"{tag}t1")
    t2 = pool.tile([16, 1], F32, tag=f"{tag}t2")
    for _ in range(3):
        g.tensor_tensor(t1[:], y[:], y[:], ALU.mult)
        g.tensor_tensor(t2[:], t1[:], x[:], ALU.mult)
        g.tensor_tensor(t2[:], t2[:], neghalf[:], ALU.mult)
        g.tensor_tensor(t2[:], t2[:], c15[:], ALU.add)
        g.tensor_tensor(y[:], y[:], t2[:], ALU.mult)
    return y


def build_nc(n_cores=NCORES):
    nc = bacc.Bacc("TRN2", target_bir_lowering=False, debug=False,
                   enable_asserts=False, num_devices=n_cores)

    xT    = nc.dram_tensor("xT",    [KE, 128, BT], F32, kind="ExternalInput").ap()
    wih0  = nc.dram_tensor("wih0",  [E, MSL], F32, kind="ExternalInput").ap()
    whh0  = nc.dram_tensor("whh0",  [H, MSL], F32, kind="ExternalInput").ap()
    wih1  = nc.dram_tensor("wih1",  [H, MSL], F32, kind="ExternalInput").ap()
    whh1  = nc.dram_tensor("whh1",  [H, MSL], F32, kind="ExternalInput").ap()
    bc0   = nc.dram_tensor("bc0",   [128, MSL], F32, kind="ExternalInput").ap()
    cin0  = nc.dram_tensor("cin0",  [16, MSL], F32, kind="ExternalInput").ap()
    cinI  = nc.dram_tensor("cinI",  [16, MSL], F32, kind="ExternalInput").ap()
    cinH  = nc.dram_tensor("cinH",  [16, MSL], F32, kind="ExternalInput").ap()
    vw0b  = nc.dram_tensor("vw0b",  [16, MSL], F32, kind="ExternalInput").ap()
    vwib  = nc.dram_tensor("vwib",  [16, MSL], F32, kind="ExternalInput").ap()
    vwhb  = nc.dram_tensor("vwhb",  [16, MSL], F32, kind="ExternalInput").ap()
    wT0b  = nc.dram_tensor("wT0b",  [16, 128], F32, kind="ExternalInput").ap()
    wT1b  = nc.dram_tensor("wT1b",  [16, 128], F32, kind="ExternalInput").ap()
    lnbT0 = nc.dram_tensor("lnbT0", [16, 128], F32, kind="ExternalInput").ap()
    lnbT1 = nc.dram_tensor("lnbT1", [16, 128], F32, kind="ExternalInput").ap()
    eye16 = nc.dram_tensor("eye16", [16, 16], F32, kind="ExternalInput").ap()
    w1sb  = nc.dram_tensor("w1sb",  [H, H], BF16, kind="ExternalInput").ap()
    w1xb  = nc.dram_tensor("w1xb",  [2, H], BF16, kind="ExternalInput").ap()
    w2sb  = nc.dram_tensor("w2sb",  [H, VC], BF16, kind="ExternalInput").ap()
    ln2w  = nc.dram_tensor("ln2w",  [128, KH], F32, kind="ExternalInput").ap()
    ln2b  = nc.dram_tensor("ln2b",  [128, KH], F32, kind="ExternalInput").ap()
    out   = nc.dram_tensor("out",   [BT, VC], F32, kind="ExternalOutput").ap()

    rg = [list(range(n_cores))]

    with tile.TileContext(nc) as tc:
        with tc.tile_pool(name="dramp", bufs=1, space="DRAM") as dramp:
            gi0d = dramp.tile([BT, MSL], F32)
            hstats = dramp.tile([BT, 2], F32)
            h1store = dramp.tile([T, n_cores, PAY], F32)

            # ============ Phase 0: gi0 = x @ Wih0_slice.T + bih0 ============
            with tc.tile_pool(name="p0", bufs=1) as p0, \
                 tc.tile_pool(name="p0o", bufs=3) as p0o, \
                 tc.tile_pool(name="ps0", bufs=2, space="PSUM") as ps0:
                xs = p0.tile([128, KE, BT], F32)
                nc.sync.dma_start(xs[:], xT.rearrange("k p n -> p k n"))
                wih0s = p0.tile([128, KE, MSL], F32)
                nc.sync.dma_start(wih0s[:],
                                  wih0.rearrange("(k p) m -> p k m", p=128))
                bc0s = p0.tile([128, MSL], F32)
                nc.sync.dma_start(bc0s[:], bc0[:])
                for blk in range(BT // 128):
                    ps = ps0.tile([128, MSL], F32, tag="ps")
                    for ke in range(KE):
                        nc.tensor.matmul(ps[:],
                                         xs[:, ke, blk * 128:(blk + 1) * 128],
                                         wih0s[:, ke, :],
                                         start=(ke == 0), stop=(ke == KE - 1))
                    gt = p0o.tile([128, MSL], F32, tag="gt")
                    nc.vector.scalar_tensor_tensor(gt[:], ps[:], 1.0, bc0s[:],
                                                   ALU.mult, ALU.add)
                    nc.sync.dma_start(gi0d[blk * 128:(blk + 1) * 128, :], gt[:])

            # ==================== Phase 1: recurrence ====================
            with tc.tile_pool(name="rw", bufs=1) as rw, \
                 tc.tile_pool(name="rs", bufs=3) as rs, \
                 tc.tile_pool(name="rt", bufs=3) as rt, \
                 tc.tile_pool(name="rk", bufs=2) as rk, \
                 tc.tile_pool(name="rps", bufs=1, space="PSUM") as rps, \
                 tc.tile_pool(name="rpt", bufs=2, space="PSUM") as rpt, \
                 tc.tile_pool(name="rdram", bufs=3, space="DRAM") as rdram:

                # ---- persistent weights / constants ----
                whh0s = rw.tile([128, KH, MSL], F32)
                nc.sync.dma_start(whh0s[:],
                                  whh0.rearrange("(k p) m -> p k m", p=128))
                wih1s = rw.tile([128, KH, MSL], F32)
                nc.sync.dma_start(wih1s[:],
                                  wih1.rearrange("(k p) m -> p k m", p=128))
                whh1s = rw.tile([128, KH, MSL], F32)
                nc.sync.dma_start(whh1s[:],
                                  whh1.rearrange("(k p) m -> p k m", p=128))
                cin0s = rw.tile([16, MSL], F32)
                nc.sync.dma_start(cin0s[:], cin0[:])
                cinIs = rw.tile([16, MSL], F32)
                nc.sync.dma_start(cinIs[:], cinI[:])
                cinHs = rw.tile([16, MSL], F32)
                nc.sync.dma_start(cinHs[:], cinH[:])
                vw0s = rw.tile([16, MSL], F32)
                nc.sync.dma_start(vw0s[:], vw0b[:])
                vwis = rw.tile([16, MSL], F32)
                nc.sync.dma_start(vwis[:], vwib[:])
                vwhs = rw.tile([16, MSL], F32)
                nc.sync.dma_start(vwhs[:], vwhb[:])
                wT0s = rw.tile([16, 128], F32)
                nc.sync.dma_start(wT0s[:], wT0b[:])
                wT1s = rw.tile([16, 128], F32)
                nc.sync.dma_start(wT1s[:], wT1b[:])
                lnb0s = rw.tile([16, 128], F32)
                nc.sync.dma_start(lnb0s[:], lnbT0[:])
                lnb1s = rw.tile([16, 128], F32)
                nc.sync.dma_start(lnb1s[:], lnbT1[:])
                eyes = rw.tile([16, 16], F32)
                nc.sync.dma_start(eyes[:], eye16[:])
                ktile = rw.tile([16, 1], I32)
                nc.vector.memset(ktile[:], QK)
                one_i = rw.tile([16, 1], I32)
                nc.vector.memset(one_i[:], 1)
                c15 = rw.tile([16, 1], F32)
                nc.vector.memset(c15[:], 1.5)
                neghalf = rw.tile([16, 1], F32)
                nc.vector.memset(neghalf[:], -0.5)
                negone = rw.tile([16, 1], F32)
                nc.vector.memset(negone[:], -1.0)
                epsT = rw.tile([16, 1], F32)
                nc.vector.memset(epsT[:], EPS)
                cts = (c15, neghalf)
                z128 = rw.tile([16, 128], F32)
                nc.vector.memset(z128[:], 0.0)

                u0T_prev = z128
                u1T_prev = z128
                u1T_prev2 = z128   # for k==1
                agoA_prev = None
                agoB_prev = None

                for k in range(T + 2):
                    lastA = (k >= T)       # no layer0 gates / producer-A
                    lastB = (k >= T + 1)   # no layer1 gates / producer-B

                    # ---------- consume AG_A[k-1] ----------
                    if k <= T:
                        u0g = rs.tile([128, KH, 16], F32, tag="u0g")
                        st0in = rt.tile([16, KH, 8], F32, tag="st0in")
                        if k == 0:
                            nc.vector.memset(u0g[:], 0.0)
                            nc.vector.memset(st0in[:], 0.0)
                            nc.vector.memset(st0in[:, :, 0], 64.0)
                            nc.vector.memset(st0in[:, :, 3], 64.0)
                        else:
                            nc.sync.dma_start(
                                u0g[:], agoA_prev[:, 0:2048].rearrange(
                                    "c (p b) -> p c b", p=128))
                            nc.scalar.dma_start(
                                st0in[:, :, 0:6],
                                agoA_prev[:, 2048:2144].rearrange(
                                    "c (b s) -> b c s", b=16))
                        # layer0 stats -> rstd0, mrs_pos0, mrs_neg0
                        agg0 = rt.tile([16, 2], F32, tag="agg0")
                        nc.vector.bn_aggr(agg0[:], st0in[:, :, 0:6])
                        ve0 = rt.tile([16, 1], F32, tag="ve0")
                        nc.gpsimd.tensor_tensor(ve0[:], agg0[:, 1:2], epsT[:], ALU.add)
                        rstd0 = _rsqrt(nc, rt, ve0, ktile, one_i, cts, "r0")
                        mrs_p0 = rt.tile([16, 1], F32, tag="mrsp0")
                        nc.vector.tensor_tensor(mrs_p0[:], agg0[:, 0:1],
                                                rstd0[:], ALU.mult)
                        mrs_n0 = rt.tile([16, 1], F32, tag="mrsn0")
                        nc.gpsimd.tensor_tensor(mrs_n0[:], mrs_p0[:], negone[:], ALU.mult)

                    # ---------- consume AG_B[k-1] ----------
                    u1g = rs.tile([128, KH, 16], F32, tag="u1g")
                    st1in = rt.tile([16, KH, 8], F32, tag="st1in")
                    if k <= 1:
                        nc.vector.memset(u1g[:], 0.0)
                        nc.vector.memset(st1in[:], 0.0)
                        nc.vector.memset(st1in[:, :, 0], 64.0)
                        nc.vector.memset(st1in[:, :, 3], 64.0)
                    else:
                        nc.sync.dma_start(
                            u1g[:], agoB_prev[:, 0:2048].rearrange(
                                "c (p b) -> p c b", p=128))
                        nc.scalar.dma_start(
                            st1in[:, :, 0:6],
                            agoB_prev[:, 2048:2144].rearrange(
                                "c (b s) -> b c s", b=16))
                        nc.sync.dma_start(h1store[k - 2], agoB_prev[:])
                    agg1 = rt.tile([16, 2], F32, tag="agg1")
                    nc.vector.bn_aggr(agg1[:], st1in[:, :, 0:6])
                    ve1 = rt.tile([16, 1], F32, tag="ve1")
                    nc.gpsimd.tensor_tensor(ve1[:], agg1[:, 1:2], epsT[:], ALU.add)
                    rstd1 = _rsqrt(nc, rt, ve1, ktile, one_i, cts, "r1")
                    mrs_p1 = rt.tile([16, 1], F32, tag="mrsp1")
                    nc.vector.tensor_tensor(mrs_p1[:], agg1[:, 0:1], rstd1[:],
                                            ALU.mult)
                    mrs_n1 = rt.tile([16, 1], F32, tag="mrsn1")
                    nc.gpsimd.tensor_tensor(mrs_n1[:], mrs_p1[:], negone[:], ALU.mult)
                    # head stats row: (-mean1, sqrt(var1+eps)) for t=k-2
                    if 2 <= k:
                        hs = rt.tile([16, 2], F32, tag="hs")
                        nc.gpsimd.tensor_tensor(hs[:, 0:1], agg1[:, 0:1],
                                                negone[:], ALU.mult)
                        nc.gpsimd.tensor_tensor(hs[:, 1:2], ve1[:], rstd1[:],
                                                ALU.mult)
                        nc.gpsimd.dma_start(
                            hstats[(k - 2) * 16:(k - 1) * 16, :], hs[:])

                    # ---------- gate matmul chains ----------
                    if not lastA:
                        pS0 = rps.tile([16, MSL], F32, tag="pS0")
                        for kk in range(KH):
                            nc.tensor.matmul(pS0[:], u0g[:, kk, :],
                                             whh0s[:, kk, :],
                                             start=(kk == 0), stop=(kk == KH - 1))
                    if not lastB:
                        pS1i = rps.tile([16, MSL], F32, tag="pS1i")
                        for kk in range(KH):
                            nc.tensor.matmul(pS1i[:], u0g[:, kk, :],
                                             wih1s[:, kk, :],
                                             start=(kk == 0), stop=(kk == KH - 1))
                        pS1h = rps.tile([16, MSL], F32, tag="pS1h")
                        for kk in range(KH):
                            nc.tensor.matmul(pS1h[:], u1g[:, kk, :],
                                             whh1s[:, kk, :],
                                             start=(kk == 0), stop=(kk == KH - 1))

                    # ---------- layer0 gates -> h0raw[k] ----------
                    if not lastA:
                        gi0c = rk.tile([16, MSL], F32, tag="gi0c")
                        nc.gpsimd.dma_start(
                            gi0c[:], gi0d[k * 16:(k + 1) * 16, :])
                        corr0 = rt.tile([16, MSL], F32, tag="corr0")
                        nc.vector.scalar_tensor_tensor(
                            corr0[:], vw0s[:], mrs_p0[:], cin0s[:],
                            ALU.mult, ALU.subtract)
                        pre0 = rt.tile([16, MSL], F32, tag="pre0")
                        nc.vector.scalar_tensor_tensor(
                            pre0[:], pS0[:], rstd0[:], corr0[:],
                            ALU.mult, ALU.subtract)
                        rz0 = rt.tile([16, 256], F32, tag="rz0")
                        nc.gpsimd.tensor_tensor(rz0[:], pre0[:, 0:256],
                                                gi0c[:, 0:256], ALU.add)
                        sg0 = rt.tile([16, 256], F32, tag="sg0")
                        nc.scalar.activation(sg0[:], rz0[:], AF.Sigmoid)
                        n0a = rt.tile([16, 128], F32, tag="n0a")
                        nc.vector.tensor_tensor(n0a[:], sg0[:, 0:128],
                                                pre0[:, 256:384], ALU.mult)
                        nc.gpsimd.tensor_tensor(n0a[:], n0a[:],
                                                gi0c[:, 256:384], ALU.add)
                        n0 = rt.tile([16, 128], F32, tag="n0")
                        nc.scalar.activation(n0[:], n0a[:], AF.Tanh)
                        # hp0 = LN0(h0raw[k-1]) own slice
                        q0 = rt.tile([16, 128], F32, tag="q0")
                        nc.vector.scalar_tensor_tensor(
                            q0[:], wT0s[:], mrs_n0[:], lnb0s[:],
                            ALU.mult, ALU.add)
                        hp0 = rt.tile([16, 128], F32, tag="hp0")
                        nc.vector.scalar_tensor_tensor(
                            hp0[:], u0T_prev[:], rstd0[:], q0[:],
                            ALU.mult, ALU.add)
                        d0 = rt.tile([16, 128], F32, tag="d0")
                        nc.gpsimd.tensor_tensor(d0[:], hp0[:], n0[:],
                                                ALU.subtract)
                        nc.vector.tensor_tensor(d0[:], d0[:], sg0[:, 128:256],
                                                ALU.mult)
                        h0n = rt.tile([16, 128], F32, tag="h0n")
                        nc.gpsimd.tensor_tensor(h0n[:], d0[:], n0[:], ALU.add)
                        # producer A
                        u0T = rk.tile([16, 128], F32, tag="u0T")
                        nc.vector.tensor_tensor(u0T[:], h0n[:], wT0s[:],
                                                ALU.mult)
                        st0 = rt.tile([16, 6], F32, tag="st0")
                        nc.vector.bn_stats(st0[:], h0n[:])
                        pt0 = rpt.tile([128, 16], F32, tag="pt0")
                        nc.tensor.transpose(pt0[:], u0T[:], eyes[:])
                        u0s = rt.tile([128, 16], F32, tag="u0s")
                        nc.scalar.copy(u0s[:], pt0[:])
                        aginA = rdram.tile([PAY], F32, tag="aginA")
                        nc.sync.dma_start(
                            aginA[0:2048].rearrange("(p b) -> p b", p=128),
                            u0s[:])
                        nc.sync.dma_start(
                            aginA[2048:2144].rearrange("(b s) -> b s", b=16),
                            st0[:])
                        agoA = rdram.tile([n_cores, PAY], F32,
                                          tag="agoA", addr_space="Shared")
                        nc.gpsimd.collective_compute(
                            "AllGather", ALU.bypass, replica_groups=rg,
                            ins=[aginA.opt()], outs=[agoA.opt()])
                        u0T_prev = u0T
                        agoA_prev = agoA

                    # ---------- layer1 gates -> h1raw[k-1] ----------
                    if not lastB:
                        cA = rt.tile([16, MSL], F32, tag="cA")
                        nc.vector.scalar_tensor_tensor(
                            cA[:], vwis[:], mrs_p0[:], cinIs[:],
                            ALU.mult, ALU.subtract)
                        gA = rt.tile([16, MSL], F32, tag="gA")
                        nc.vector.scalar_tensor_tensor(
                            gA[:], pS1i[:], rstd0[:], cA[:],
                            ALU.mult, ALU.subtract)
                        cB = rt.tile([16, MSL], F32, tag="cB")
                        nc.vector.scalar_tensor_tensor(
                            cB[:], vwhs[:], mrs_p1[:], cinHs[:],
                            ALU.mult, ALU.subtract)
                        gB = rt.tile([16, MSL], F32, tag="gB")
                        nc.vector.scalar_tensor_tensor(
                            gB[:], pS1h[:], rstd1[:], cB[:],
                            ALU.mult, ALU.subtract)
                        rz1 = rt.tile([16, 256], F32, tag="rz1")
                        nc.gpsimd.tensor_tensor(rz1[:], gA[:, 0:256],
                                                gB[:, 0:256], ALU.add)
                        sg1 = rt.tile([16, 256], F32, tag="sg1")
                        nc.scalar.activation(sg1[:], rz1[:], AF.Sigmoid)
                        n1a = rt.tile([16, 128], F32, tag="n1a")
                        nc.vector.tensor_tensor(n1a[:], sg1[:, 0:128],
                                                gB[:, 256:384], ALU.mult)
                        nc.gpsimd.tensor_tensor(n1a[:], n1a[:],
                                                gA[:, 256:384], ALU.add)
                        n1 = rt.tile([16, 128], F32, tag="n1")
                        nc.scalar.activation(n1[:], n1a[:], AF.Tanh)
                        q1 = rt.tile([16, 128], F32, tag="q1")
                        nc.vector.scalar_tensor_tensor(
                            q1[:], wT1s[:], mrs_n1[:], lnb1s[:],
                            ALU.mult, ALU.add)
                        hp1 = rt.tile([16, 128], F32, tag="hp1")
                        up1 = u1T_prev2 if k == 1 else u1T_prev
                        nc.vector.scalar_tensor_tensor(
                            hp1[:], up1[:], rstd1[:], q1[:],
                            ALU.mult, ALU.add)
                        d1 = rt.tile([16, 128], F32, tag="d1")
                        nc.gpsimd.tensor_tensor(d1[:], hp1[:], n1[:],
                                                ALU.subtract)
                        nc.vector.tensor_tensor(d1[:], d1[:], sg1[:, 128:256],
                                                ALU.mult)
                        h1n = rt.tile([16, 128], F32, tag="h1n")
                        nc.gpsimd.tensor_tensor(h1n[:], d1[:], n1[:], ALU.add)
                        u1T = rk.tile([16, 128], F32, tag="u1T")
                        nc.vector.tensor_tensor(u1T[:], h1n[:], wT1s[:],
                                                ALU.mult)
                        st1 = rt.tile([16, 6], F32, tag="st1")
                        nc.vector.bn_stats(st1[:], h1n[:])
                        pt1 = rpt.tile([128, 16], F32, tag="pt1")
                        nc.tensor.transpose(pt1[:], u1T[:], eyes[:])
                        u1s = rt.tile([128, 16], F32, tag="u1s")
                        nc.scalar.copy(u1s[:], pt1[:])
                        aginB = rdram.tile([PAY], F32, tag="aginB")
                        nc.sync.dma_start(
                            aginB[0:2048].rearrange("(p b) -> p b", p=128),
                            u1s[:])
                        nc.sync.dma_start(
                            aginB[2048:2144].rearrange("(b s) -> b s", b=16),
                            st1[:])
                        agoB = rdram.tile([n_cores, PAY], F32,
                                          tag="agoB", addr_space="Shared")
                        nc.gpsimd.collective_compute(
                            "AllGather", ALU.bypass, replica_groups=rg,
                            ins=[aginB.opt()], outs=[agoB.opt()])
                        u1T_prev = u1T
                        agoB_prev = agoB

            # ==================== Phase 2: head (bf16) ====================
            with tc.tile_pool(name="hw", bufs=1) as hw, \
                 tc.tile_pool(name="hh", bufs=1) as hh, \
                 tc.tile_pool(name="ht", bufs=1) as ht, \
                 tc.tile_pool(name="ho", bufs=2) as ho, \
                 tc.tile_pool(name="hp1p", bufs=2, space="PSUM") as hp1p, \
                 tc.tile_pool(name="hp2p", bufs=2, space="PSUM") as hp2p, \
                 tc.tile_pool(name="hps", bufs=1, space="PSUM") as hps:
                w1s = hw.tile([128, KH, H], BF16)
                nc.sync.dma_start(w1s[:],
                                  w1sb.rearrange("(k p) m -> p k m", p=128))
                w1x = hw.tile([2, H], BF16)
                nc.sync.dma_start(w1x[:], w1xb[:])
                w2s = hw.tile([128, KH, VC], BF16)
                nc.sync.dma_start(w2s[:],
                                  w2sb.rearrange("(k p) v -> p k v", p=128))
                ln2ws = hw.tile([128, KH], F32)
                nc.sync.dma_start(ln2ws[:], ln2w[:])
                ln2bs = hw.tile([128, KH], F32)
                nc.sync.dma_start(ln2bs[:], ln2b[:])
                ones_col = hw.tile([128, 1], F32)
                nc.vector.memset(ones_col[:], 1.0)
                ones_row = hw.tile([1, 128], F32)
                nc.vector.memset(ones_row[:], 1.0)

                NB = BT // 512   # 8 batches of 512 cols (32 steps)
                for j in range(NB):
                    t0 = j * 32
                    # u1 batch [128, KH, 512] from agoutB[t+1], cast bf16
                    hk = hh.tile([128, KH, 32, 16], F32, tag="hk")
                    for c in range(KH):
                        nc.sync.dma_start(
                            hk[:, c], h1store[t0:t0 + 32, c, 0:2048].rearrange(
                                "t (p b) -> p t b", p=128))
                    hkb = hh.tile([128, KH, 32, 16], BF16, tag="hkb")
                    nc.vector.tensor_copy(hkb[:, 0:4], hk[:, 0:4])
                    nc.scalar.copy(hkb[:, 4:8], hk[:, 4:8])
                    hst = ht.tile([2, 512], F32, tag="hst")
                    nc.scalar.dma_start(
                        hst[:], hstats[t0 * 16:(t0 + 32) * 16, :].rearrange(
                            "n s -> s n"))
                    hstb = ht.tile([2, 512], BF16, tag="hstb")
                    nc.vector.tensor_copy(hstb[:], hst[:])

                    # W1 + folded LN1 -> leaky -> LN2 stats
                    atiles = []
                    ps_s = hps.tile([1, 512], F32, tag="ps_s")
                    ps_q = hps.tile([1, 512], F32, tag="ps_q")
                    for m in range(KH):
                        ps_a = hp1p.tile([128, 512], F32, tag="ps_a")
                        for kk in range(KH):
                            nc.tensor.matmul(ps_a[:],
                                             w1s[:, kk, m * 128:(m + 1) * 128],
                                             hkb[:, kk, :],
                                             start=(kk == 0), stop=False)
                        nc.tensor.matmul(ps_a[:], w1x[:, m * 128:(m + 1) * 128],
                                         hstb[:], start=False, stop=True)
                        rl = ho.tile([128, 512], F32, tag="rl")
                        nc.scalar.activation(rl[:], ps_a[:], AF.Relu)
                        at = ht.tile([128, 512], F32, tag=f"at{m}")
                        # leaky = alpha*(y - relu(y)) + relu(y)
                        nc.vector.scalar_tensor_tensor(
                            at[:], ps_a[:], 1.0, rl[:], ALU.mult, ALU.subtract)
                        nc.vector.scalar_tensor_tensor(
                            at[:], at[:], NEG_SLOPE, rl[:], ALU.mult, ALU.add)
                        atiles.append(at)
                        nc.tensor.matmul(ps_s[:], ones_col[:], at[:],
                                         start=(m == 0), stop=(m == KH - 1))
                        sq = ho.tile([128, 512], F32, tag="sq")
                        nc.scalar.activation(sq[:], at[:], AF.Square)
                        nc.tensor.matmul(ps_q[:], ones_col[:], sq[:],
                                         start=(m == 0), stop=(m == KH - 1))
                    mean2 = ht.tile([1, 512], F32, tag="mean2")
                    nc.vector.tensor_scalar_mul(mean2[:], ps_s[:], 1.0 / H)
                    var2 = ht.tile([1, 512], F32, tag="var2")
                    nc.vector.tensor_tensor(var2[:], mean2[:], mean2[:],
                                            ALU.mult)
                    nc.vector.scalar_tensor_tensor(var2[:], ps_q[:], 1.0 / H,
                                                   var2[:], ALU.mult,
                                                   ALU.subtract)
                    nc.vector.tensor_scalar_add(var2[:], var2[:], EPS)
                    nc.vector.reciprocal(var2[:], var2[:])
                    rstd2 = ht.tile([1, 512], F32, tag="rstd2")
                    nc.scalar.activation(rstd2[:], var2[:], AF.Sqrt)
                    ps_mb = hps.tile([128, 512], F32, tag="ps_mb")
                    nc.tensor.matmul(ps_mb[:], ones_row[:], mean2[:],
                                     start=True, stop=True)
                    ps_rb = hps.tile([128, 512], F32, tag="ps_rb")
                    nc.tensor.matmul(ps_rb[:], ones_row[:], rstd2[:],
                                     start=True, stop=True)
                    ab = hh.tile([128, KH, 512], BF16, tag="ab")
                    for m in range(KH):
                        an = ho.tile([128, 512], F32, tag="an")
                        nc.vector.tensor_tensor(an[:], atiles[m][:], ps_mb[:],
                                                ALU.subtract)
                        nc.vector.tensor_tensor(an[:], an[:], ps_rb[:],
                                                ALU.mult)
                        nc.vector.tensor_scalar(ab[:, m], an[:],
                                                ln2ws[:, m:m + 1],
                                                ln2bs[:, m:m + 1],
                                                ALU.mult, ALU.add)

                    # W2: out[cols, VC]
                    for cc in range(4):
                        for vs in range(8):
                            pv = hp2p.tile([128, 500], F32, tag="pv")
                            for kk in range(KH):
                                nc.tensor.matmul(
                                    pv[:],
                                    ab[:, kk, cc * 128:(cc + 1) * 128],
                                    w2s[:, kk, vs * 500:(vs + 1) * 500],
                                    start=(kk == 0), stop=(kk == KH - 1))
                            ot = ho.tile([128, 500], F32, tag="ot")
                            if vs % 2 == 0:
                                nc.vector.tensor_copy(ot[:], pv[:])
                            else:
                                nc.scalar.copy(ot[:], pv[:])
                            nc.sync.dma_start(
                                out[(t0 * 16) + cc * 128:
                                    (t0 * 16) + (cc + 1) * 128,
                                    vs * 500:(vs + 1) * 500], ot[:])
    return nc


# ===================== host-side prep / post =====================

def _np(x):
    return np.asarray(x)


def _bf16(x):
    x = np.ascontiguousarray(x, np.float32)
    u = x.view(np.uint32)
    r = ((u >> 16) + ((u >> 15) & 1)).astype(np.uint32) << 16
    return r.view(np.float32)


def prep_in_maps(inputs, n_cores=NCORES):
    import ml_dtypes
    ids = _np(inputs['input']).astype(np.int64)[:, :T]
    embd = _np(inputs['embd']).astype(np.float32)
    x = embd[ids]                                               # [B, T, E]
    xT = np.ascontiguousarray(x.transpose(2, 1, 0).reshape(E, BT))
    xT = xT.reshape(KE, 128, BT)

    def gate_slice(W, c):
        cols = []
        for g in range(3):
            cols.append(W[g * H + c * 128:(g * H + (c + 1) * 128), :])
        Wc = np.concatenate(cols, axis=0)                        # [384, D]
        return np.ascontiguousarray(Wc.T)                        # [D, 384]

    def vec_slice(v, c):
        return np.concatenate([v[g * H + c * 128:g * H + (c + 1) * 128]
                               for g in range(3)])               # [384]

    Wih0 = _np(inputs['Wih0']).astype(np.float32)
    Whh0 = _np(inputs['Whh0']).astype(np.float32)
    Wih1 = _np(inputs['Wih1']).astype(np.float32)
    Whh1 = _np(inputs['Whh1']).astype(np.float32)
    bih0 = _np(inputs['bih0']).astype(np.float32)
    bhh0 = _np(inputs['bhh0']).astype(np.float32)
    bih1 = _np(inputs['bih1']).astype(np.float32)
    bhh1 = _np(inputs['bhh1']).astype(np.float32)
    lnw0 = _np(inputs['ln0_w']).astype(np.float32)
    lnb0 = _np(inputs['ln0_b']).astype(np.float32)
    lnw1 = _np(inputs['ln1_w']).astype(np.float32)
    lnb1 = _np(inputs['ln1_b']).astype(np.float32)
    W1 = _np(inputs['W1']).astype(np.float32)
    b1 = _np(inputs['b1']).astype(np.float32)
    W2 = _np(inputs['W2']).astype(np.float32)

    w1T = np.ascontiguousarray(W1.T)                             # [H(k), H(j)]
    vw1 = lnw1 @ w1T                                             # [H]
    vb1 = lnb1 @ w1T + b1                                        # [H]
    w1sb = w1T.astype(ml_dtypes.bfloat16)
    w1xb = np.stack([vw1, vb1]).astype(ml_dtypes.bfloat16)       # [2, H]
    ln2w_t = np.ascontiguousarray(
        _np(inputs['ln2_w']).astype(np.float32).reshape(KH, 128).T)
    ln2b_t = np.ascontiguousarray(
        _np(inputs['ln2_b']).astype(np.float32).reshape(KH, 128).T)
    eye = np.eye(16, dtype=np.float32)

    in_maps = []
    for c in range(n_cores):
        whh0c = gate_slice(Whh0, c)                              # [H, 384]
        wih1c = gate_slice(Wih1, c)
        whh1c = gate_slice(Whh1, c)
        wih0c = gate_slice(Wih0, c)                              # [E, 384]
        vw0 = lnw0 @ whh0c                                       # [384]
        vb0 = lnb0 @ whh0c
        vwi = lnw0 @ wih1c
        vbi = lnb0 @ wih1c
        vwh = lnw1 @ whh1c
        vbh = lnb1 @ whh1c
        bc0 = np.tile(vec_slice(bih0, c)[None, :], (128, 1))     # [128, 384]
        cin0 = np.tile((vec_slice(bhh0, c) + vb0)[None, :], (16, 1))
        cinI = np.tile((vec_slice(bih1, c) + vbi)[None, :], (16, 1))
        cinH = np.tile((vec_slice(bhh1, c) + vbh)[None, :], (16, 1))
        wT0b = np.tile(lnw0[c * 128:(c + 1) * 128][None, :], (16, 1))
        wT1b = np.tile(lnw1[c * 128:(c + 1) * 128][None, :], (16, 1))
        lnbT0 = np.tile(lnb0[c * 128:(c + 1) * 128][None, :], (16, 1))
        lnbT1 = np.tile(lnb1[c * 128:(c + 1) * 128][None, :], (16, 1))
        w2cT = np.ascontiguousarray(
            W2[c * VC:(c + 1) * VC, :].T).astype(ml_dtypes.bfloat16)
        in_maps.append({
            'xT': xT, 'wih0': wih0c, 'whh0': whh0c,
            'wih1': wih1c, 'whh1': whh1c,
            'bc0': np.ascontiguousarray(bc0, np.float32),
            'cin0': np.ascontiguousarray(cin0, np.float32),
            'cinI': np.ascontiguousarray(cinI, np.float32),
            'cinH': np.ascontiguousarray(cinH, np.float32),
            'vw0b': np.ascontiguousarray(np.tile(vw0[None, :], (16, 1)),
                                         np.float32),
            'vwib': np.ascontiguousarray(np.tile(vwi[None, :], (16, 1)),
                                         np.float32),
            'vwhb': np.ascontiguousarray(np.tile(vwh[None, :], (16, 1)),
                                         np.float32),
            'wT0b': np.ascontiguousarray(wT0b, np.float32),
            'wT1b': np.ascontiguousarray(wT1b, np.float32),
            'lnbT0': np.ascontiguousarray(lnbT0, np.float32),
            'lnbT1': np.ascontiguousarray(lnbT1, np.float32),
            'eye16': eye,
            'w1sb': np.ascontiguousarray(w1sb),
            'w1xb': np.ascontiguousarray(w1xb),
            'w2sb': np.ascontiguousarray(w2cT),
            'ln2w': ln2w_t, 'ln2b': ln2b_t,
        })
    return in_maps


def postprocess(results, inputs):
    b2 = _np(inputs['b2']).astype(np.float32)
    full = np.concatenate([r['out'] for r in results], axis=1)   # [BT, V]
    full = full.reshape(T, B, V).transpose(1, 0, 2)              # [B, T, V]
    return full + b2


# ===================== NEFF disk cache =====================

def _install_neff_cache():
    import hashlib, os, shutil
    import concourse.bass2jax as b2j
    from concourse.bass_utils import compile_bir_kernel as _real
    if getattr(b2j, "_ant_neff_cache_installed", False):
        return
    cache_dir = os.path.expanduser("~/.cache/bass_neff_cache")
    os.makedirs(cache_dir, exist_ok=True)

    def cached(bir_json, tmpdir, neff_name="file.neff"):
        key = hashlib.sha256(bir_json).hexdigest()
        p = os.path.join(cache_dir, key + ".neff")
        out = os.path.join(tmpdir, neff_name)
        if os.path.exists(p):
            shutil.copyfile(p, out)
            return out
        r = _real(bir_json, tmpdir, neff_name)
        try:
            shutil.copyfile(r, p)
        except OSError:
            pass
        return r

    b2j.compile_bir_kernel = cached
    b2j._ant_neff_cache_installed = True


# ===================== NTFF profile shim (for traced runs) ==================

def _install_axon_prof():
    import types, ctypes, contextlib
    try:
        from antenv import axon_hooks  # noqa: F401
        return
    except ImportError:
        pass
    so_path = "/opt/axon/libaxon_pjrt.so"
    try:
        lib = ctypes.CDLL(so_path)
    except OSError:
        return
    hook = None
    if hasattr(lib, "axon_start_nrt_profile"):
        lib.axon_start_nrt_profile.argtypes = [
            ctypes.POINTER(ctypes.c_int64), ctypes.c_size_t]
        lib.axon_start_nrt_profile.restype = ctypes.c_int64
        lib.axon_stop_nrt_profile.argtypes = [ctypes.c_char_p]
        lib.axon_stop_nrt_profile.restype = ctypes.c_int64

        @contextlib.contextmanager
        def hook(output_dir, device_ids):
            import jax
            jax.devices()
            if device_ids:
                ids = (ctypes.c_int64 * len(device_ids))(*device_ids)
                rc = lib.axon_start_nrt_profile(ids, len(device_ids))
            else:
                rc = lib.axon_start_nrt_profile(None, 0)
            if rc != 0:
                raise RuntimeError(f"axon_start_nrt_profile rc={rc}")
            try:
                yield
            finally:
                lib.axon_stop_nrt_profile(str(output_dir).encode())

    mod = types.ModuleType("antenv.axon_hooks")
    _h = [hook]
    mod.set_axon_ntff_profile_hook = lambda h: _h.__setitem__(0, h)
    mod.get_axon_ntff_profile_hook = lambda: _h[0]
    _sys.modules["antenv.axon_hooks"] = mod
    import antenv
    antenv.axon_hooks = mod


# ===================== entry point =====================

_NC = None


def _get_nc():
    global _NC
    if _NC is None:
        _install_neff_cache()
        nc = build_nc()
        nc.compile()
        _NC = nc
    return _NC


def kernel(**inputs):
    from concourse import bass_utils
    nc = _get_nc()
    in_maps = prep_in_maps(inputs)
    res = bass_utils.run_bass_kernel_spmd(
        nc, in_maps, core_ids=list(range(NCORES)))
    return postprocess(res.results, inputs)


def kernel_traced(**inputs):
    """Like kernel() but also returns neuron-profile exec_time_ns."""
    from concourse import bass_utils
    _install_axon_prof()
    nc = _get_nc()
    in_maps = prep_in_maps(inputs)
    res = bass_utils.run_bass_kernel_spmd(
        nc, in_maps, core_ids=list(range(NCORES)), trace=True)
    return postprocess(res.results, inputs), res.exec_time_ns
